# revision 26
# baseline (speedup 1.0000x reference)
"""DigitCaps routing kernel for Trainium2 (8 NeuronCores, SPMD data-parallel over batch).

Math (per batch element b):
  u_hat[r, c, o] = sum_i W[r, c, o, i] * x[r, i]
  b_log = 0
  repeat 3x:
    c = softmax(b_log, axis=c)
    s[c, o] = sum_r c[r, c] * u_hat[r, c, o]
    v = squash(s)                               (over o)
    b_log += sum_o u_hat[r, c, o] * v[c, o]     (first 2 iters only)
  return v

Layout strategy per core (B_local = 32 batches):
  - r split into 72 supergroups (rg) of 16 (r16); phase A contracts K =
    (r16, i) = 128 using a host-built block-diagonal x as the stationary
    operand: lhsT[(r16, i), (r16', b8)] = x[b, r, i] * delta(r16, r16').
  - u_hat lives in SBUF as bf16 [(r16, b8)=128 partitions, (bg=4, rg=72, o=16, c=11)].
    Free order is (o, c) — c innermost — so the agreement's o-reduction runs as
    an in-place binary tree of tensor_tensor adds, every level of which keeps
    innermost stride-1 bf16 operands and therefore the DVE 2x perf mode
    (TensorReduce has no fast mode; the tree is ~1.5x faster).
  - s-step: y = c_sm (.) u_hat broadcast over o (middle dim, stride 0; innermost
    c stays stride-1 so the mul is 2x — no pair-duplication needed), then the
    r16-partition sum via a constant block-diagonal-ones stationary matmul.
  - agreement: z = u_hat (.) v_t (2x), then the o-tree; v_t is v replicated
    across the 16 r16 partition groups via a tiny PE matmul.
  - phase-A PSUM drains all run on ACT, keeping DVE free so agreement-0
    overlaps phase A; the iter-0 head (s0 from compact x) is emitted before
    phase A so vts0 is ready early.
"""

import sys
for p in ("/opt/trn_rl_repo", "/root/.axon_site/_ro/trn_rl_repo"):
    if p not in sys.path:
        sys.path.insert(0, p)

import numpy as np
import ml_dtypes
from contextlib import ExitStack, nullcontext

import concourse.bass as bass
import concourse.tile as tile
from concourse import bacc, mybir
from concourse.bass_utils import run_bass_kernel_spmd

# problem constants
B, R, C, I, O = 256, 1152, 11, 8, 16
ITERS = 3
EPS = 1e-9

N_CORES = 8
BL = B // N_CORES          # 32 batches per core
RG = R // 16               # 72 supergroups of 16 r
CO = C * O                 # 176 (free order (o, c): f = o*C + c)
NBG = BL // 8              # 4 b-groups of 8
NT = RG * NBG              # 288 phase-A tiles
STG = 8                    # phase-A tiles per staging DMA
ACH = 24                   # agreement rg chunk
SCH = 18                   # s-step rg chunk
F32 = mybir.dt.float32
BF16 = mybir.dt.float16
BF16_NP = np.float16


def _build_program(reps=1, abl=0):
    nc = bacc.Bacc("TRN2", target_bir_lowering=False, debug=False, num_devices=N_CORES)

    xbd_d = nc.dram_tensor("xbd", [NT // STG, 128, STG * 128], BF16, kind="ExternalInput").ap()
    wt_d = nc.dram_tensor("wt", [128, RG * CO], BF16, kind="ExternalInput").ap()
    xc_d = nc.dram_tensor("xc", [128, RG * BL], BF16, kind="ExternalInput").ap()
    ones_d = nc.dram_tensor("onesbd", [NBG, 128, 32], BF16, kind="ExternalInput").ap()
    rep_d = nc.dram_tensor("rep", [NBG, 32, 128], F32, kind="ExternalInput").ap()
    vout_d = nc.dram_tensor("vout", [BL, CO], F32, kind="ExternalOutput").ap()

    with tile.TileContext(nc) as tc, ExitStack() as ctx:
        const_p = ctx.enter_context(tc.tile_pool(name="const", bufs=1))
        stg_p = ctx.enter_context(tc.tile_pool(name="stg", bufs=3))
        uh_p = ctx.enter_context(tc.tile_pool(name="uh", bufs=1))
        log_p = ctx.enter_context(tc.tile_pool(name="logit", bufs=1))
        sm_p = ctx.enter_context(tc.tile_pool(name="smx", bufs=1))
        z_p = ctx.enter_context(tc.tile_pool(name="z", bufs=1))
        y_p = ctx.enter_context(tc.tile_pool(name="y", bufs=2))
        vt_p = ctx.enter_context(tc.tile_pool(name="vt", bufs=1))
        sq_p = ctx.enter_context(tc.tile_pool(name="sq", bufs=1))
        psA = ctx.enter_context(tc.tile_pool(name="psA", bufs=2, space=bass.MemorySpace.PSUM))
        psS = ctx.enter_context(tc.tile_pool(name="psS", bufs=2, space=bass.MemorySpace.PSUM))

        xc_sb = const_p.tile([128, RG * BL], BF16)
        nc.sync.dma_start(xc_sb[:], xc_d[:])
        w_sb = const_p.tile([128, RG * CO], BF16)
        # split into quarters so early consumers (ps0 rg-chunks, first phase-A
        # matmuls) unblock before the full 25KB/partition transfer lands
        WQ = RG // 4 * CO
        for q in range(4):
            nc.sync.dma_start(w_sb[:, q * WQ:(q + 1) * WQ], wt_d[:, q * WQ:(q + 1) * WQ])
        ones_sb = const_p.tile([128, NBG * 32], BF16)
        nc.sync.dma_start(
            ones_sb[:].rearrange("p (g m) -> p g m", g=NBG),
            ones_d[:].transpose([1, 0, 2]),
        )
        ones3 = ones_sb[:].rearrange("p (g m) -> p g m", g=NBG)
        rep_sb = const_p.tile([32, NBG * 128], F32)
        nc.sync.dma_start(
            rep_sb[:].rearrange("p (g m) -> p g m", g=NBG),
            rep_d[:].transpose([1, 0, 2]),
        )

        for _rep in range(1):
          with (tc.For_i(0, reps, 1) if reps > 1 else nullcontext()):
            # u_hat free layout: (bg, rg, o, c)
            u_hat = uh_p.tile([128, NBG * RG * CO], BF16)
            uh4 = u_hat[:].rearrange("p (g r f) -> p g r f", g=NBG, r=RG)
            uh5 = u_hat[:].rearrange("p (g r o c) -> p g r o c", g=NBG, r=RG, o=O)

            # logits: bf16, one tile per agreement pass; summed before softmax-2
            b0 = log_p.tile([128, NBG * RG * C], BF16)
            b1 = log_p.tile([128, NBG * RG * C], BF16)
            bl0 = b0[:].rearrange("p (g r c) -> p g r c", g=NBG, r=RG)
            bl1 = b1[:].rearrange("p (g r c) -> p g r c", g=NBG, r=RG)

            exp_sb = sm_p.tile([128, NBG * RG * C], BF16)
            rsum = sm_p.tile([128, NBG * RG], F32)
            rrec = sm_p.tile([128, NBG * RG], F32)
            rrec11 = sm_p.tile([128, NBG * RG * C], BF16)
            c_sm = sm_p.tile([128, NBG * RG * C], BF16)

            s_sb = sq_p.tile([32, CO], F32)
            sqv = sq_p.tile([32, CO], F32)
            ss = sq_p.tile([32, C], F32)
            t2 = sq_p.tile([32, C], F32)
            sqr = sq_p.tile([32, C], F32)
            rf = sq_p.tile([32, C], F32)
            fac = sq_p.tile([32, C], F32)
            v_sb = sq_p.tile([32, CO], F32)
            eps_t = sq_p.tile([32, 1], F32)
            nc.vector.memset(eps_t[:], EPS)
            one_t = sq_p.tile([32, 1], F32)
            nc.vector.memset(one_t[:], 1.0)
            lnsc_t = sq_p.tile([32, 1], F32)
            nc.vector.memset(lnsc_t[:], float(np.log(1.0 / C)))
            zero_t = sq_p.tile([32, 1], F32)
            nc.vector.memset(zero_t[:], 0.0)
            # preload the natural_log_exp table (Ln/Exp/Copy/Square in one set)
            # during the initial DMA wait — all later ACT ops stay in-set
            nc.scalar.activation(sqr[:], eps_t[:].broadcast_to([32, C]),
                                 mybir.ActivationFunctionType.Ln, bias=zero_t[:])

            def squash(scale):
                # v_sb = squash(s_sb * scale) over o; all tiny [32, *] ops.
                # fac = |t|/(1+|t|^2), t = scale*s: computed via Ln/Exp so ACT
                # never switches activation-table sets (Sqrt lives elsewhere):
                # fac = exp(0.5*ln(ss+eps) - ln(1+ss) + ln(scale))
                nc.vector.tensor_mul(sqv[:], s_sb[:], s_sb[:])
                nc.vector.tensor_reduce(
                    ss[:], sqv[:].rearrange("p (o c) -> p c o", o=O),
                    axis=mybir.AxisListType.X, op=mybir.AluOpType.add,
                )
                if scale != 1.0:
                    # ss holds |s_unscaled|^2; rescale to |scale*s|^2
                    nc.vector.tensor_scalar_mul(ss[:], ss[:], scale * scale)
                nc.scalar.activation(sqr[:], ss[:], mybir.ActivationFunctionType.Ln, bias=eps_t[:])
                nc.scalar.activation(t2[:], ss[:], mybir.ActivationFunctionType.Ln, bias=one_t[:])
                nc.vector.scalar_tensor_tensor(
                    rf[:], sqr[:], 0.5, t2[:],
                    op0=mybir.AluOpType.mult, op1=mybir.AluOpType.subtract,
                )
                nc.scalar.activation(fac[:], rf[:], mybir.ActivationFunctionType.Exp,
                                     bias=lnsc_t[:] if scale != 1.0 else zero_t[:])
                nc.vector.tensor_mul(
                    v_sb[:].rearrange("p (o c) -> p o c", o=O),
                    s_sb[:].rearrange("p (o c) -> p o c", o=O),
                    fac[:].unsqueeze(1).broadcast_to([32, O, C]),
                )

            def replicate_v():
                # vt_bg[(r16, b8), (o,c)] = v[bg*8 + b8, (o,c)], via PE with the
                # constant 0/1 replication matrix (f32 matmul, K=32).
                vts = []
                for bg in range(NBG):
                    ps = psA.tile([128, 256], F32, tag="psR", bufs=2)
                    nc.tensor.matmul(
                        ps[:, :CO],
                        rep_sb[:].rearrange("p (g m) -> p g m", g=NBG)[:, bg, :],
                        v_sb[:],
                        start=True, stop=True,
                    )
                    vt = vt_p.tile([128, CO], BF16, tag=f"vt{bg}")
                    nc.scalar.activation(vt[:], ps[:, :CO], mybir.ActivationFunctionType.Copy)
                    vts.append(vt)
                return vts

            def agree_chunk(eng, tag, vts, bl, bg, r0, ach):
                z = z_p.tile([128, ach * CO], BF16, tag=tag)
                eng.tensor_mul(
                    z[:].rearrange("p (r f) -> p r f", r=ach),
                    uh4[:, bg, r0:r0 + ach, :],
                    vts[bg][:].unsqueeze(1).broadcast_to([128, ach, CO]),
                )
                zoc = z[:].rearrange("p (r o c) -> p r o c", r=ach, o=O)
                for w in (8, 4, 2):
                    eng.tensor_add(
                        zoc[:, :, 0:w, :], zoc[:, :, 0:w, :], zoc[:, :, w:2 * w, :]
                    )
                eng.tensor_add(
                    bl[:, bg, r0:r0 + ach, :], zoc[:, :, 0, :], zoc[:, :, 1, :]
                )

            def agreement(vts, bl, ach=ACH):
                # bl[b, r, c] = sum_o u_hat[b, r, o, c-major] * v_t; the o-sum
                # runs as an in-place binary tree (all levels 2x on DVE).
                # rg-band-major: all 4 bg-chunks of a band unblock together as
                # soon as that band's phase-A drains land.
                for r0 in range(0, RG, ach):
                    for bg in range(NBG):
                        agree_chunk(nc.vector, f"z{ach}", vts, bl, bg, r0, ach)

            def softmax(src):
                # split by rg-halves: the first exp only needs rg<36 logits, so
                # (band-major agreements) it overlaps the agreement tail; the
                # second exp overlaps the first rsum on DVE
                G = NBG * RG
                src4 = src[:].rearrange("p (g r c) -> p g r c", g=NBG, r=RG)
                exp4 = exp_sb[:].rearrange("p (g r c) -> p g r c", g=NBG, r=RG)
                rs3 = rsum[:].rearrange("p (g r) -> p g r", g=NBG)
                RH = RG // 2
                for h in range(2):
                    nc.scalar.activation(
                        exp4[:, :, h * RH:(h + 1) * RH, :],
                        src4[:, :, h * RH:(h + 1) * RH, :],
                        mybir.ActivationFunctionType.Exp,
                    )
                rr3 = rrec[:].rearrange("p (g r) -> p g r", g=NBG)
                rd4 = rrec11[:].rearrange("p (g r c) -> p g r c", g=NBG, r=RG)
                for h in range(2):
                    nc.vector.tensor_reduce(
                        rs3[:, :, h * RH:(h + 1) * RH],
                        exp4[:, :, h * RH:(h + 1) * RH, :],
                        axis=mybir.AxisListType.X, op=mybir.AluOpType.add,
                    )
                    nc.vector.reciprocal(rr3[:, :, h * RH:(h + 1) * RH],
                                         rs3[:, :, h * RH:(h + 1) * RH])
                    # ACT writes the c-duplicated bf16 reciprocal while DVE
                    # reduces the other half; the c_sm mul then runs all
                    # stride-1 bf16 (2x) instead of a stride-0 broadcast (1x)
                    nc.scalar.activation(
                        rd4[:, :, h * RH:(h + 1) * RH, :],
                        rr3[:, :, h * RH:(h + 1) * RH].unsqueeze(3)
                            .broadcast_to([128, NBG, RH, C]),
                        mybir.ActivationFunctionType.Copy,
                    )
                nc.vector.tensor_mul(c_sm[:], exp_sb[:], rrec11[:])

            def s_step():
                # s = sum_r c_sm * u_hat: DVE premultiply (broadcast over o keeps
                # innermost c stride-1 -> 2x) + block-diag-ones matmuls; all 4
                # b-groups accumulate into one [32, 2*CO] PSUM tile (2-rg fold).
                cs3 = c_sm[:].rearrange("p (g r c) -> p g r c", g=NBG, r=RG)
                ps = psS.tile([32, 2 * CO], F32, tag="psS")
                for bg in range(NBG):
                    for r0 in range(0, RG, SCH):
                        y = y_p.tile([128, SCH * CO], BF16, tag="y")
                        nc.vector.tensor_mul(
                            y[:].rearrange("p (r o c) -> p r o c", r=SCH, o=O),
                            uh5[:, bg, r0:r0 + SCH, :, :],
                            cs3[:, bg, r0:r0 + SCH, :].unsqueeze(2)
                                .broadcast_to([128, SCH, O, C]),
                        )
                        for j in range(0, SCH, 2):
                            rg = r0 + j
                            nc.tensor.matmul(
                                ps[:], ones3[:, bg, :], y[:, j * CO:(j + 2) * CO],
                                start=(bg == 0 and rg == 0),
                                stop=(bg == NBG - 1 and rg == RG - 2),
                            )
                # both halves live in PSUM; TT can't read two PSUM operands
                nc.scalar.activation(s_sb[:], ps[:, :CO], mybir.ActivationFunctionType.Copy)
                nc.vector.tensor_add(s_sb[:], s_sb[:], ps[:, CO:])

            # ---------------- iter 0 head: s0 from compact x ----------------
            # c is uniform (1/11) in iter 0, so s0 = (1/11) sum_r u_hat is
            # computed from x and W before u_hat exists; emitted before phase A
            # so iter-0's agreement (DVE) overlaps phase A (PE/ACT/DMA).
            ps0 = psS.tile([32, 2 * CO], F32, tag="psS")
            for rg in range(RG):
                nc.tensor.matmul(
                    ps0[:, :CO], xc_sb[:, rg * BL:(rg + 1) * BL],
                    w_sb[:, rg * CO:(rg + 1) * CO],
                    start=(rg == 0), stop=(rg == RG - 1),
                )
            nc.vector.tensor_copy(s_sb[:], ps0[:, :CO])
            squash(1.0 / C)
            vts0 = replicate_v()

            # ---------------- phase A: u_hat ----------------
            # 4 matmuls (one per bg) land in a 2-bank psum tile at offsets
            # 0/176/512/688; one batched drain per rg (on ACT) writes all 4.
            PSOFF = (0, 176, 512, 688)
            for t in range(NT // STG):
                stg = stg_p.tile([128, STG * 128], BF16)
                nc.sync.dma_start(stg[:], xbd_d[t])
                for rg2 in range(2):
                    rg = t * 2 + rg2
                    ps = psA.tile([128, 1024], F32, tag="ps")
                    for bg in range(NBG):
                        k = rg2 * NBG + bg
                        nc.tensor.matmul(
                            ps[:, PSOFF[bg]:PSOFF[bg] + CO],
                            stg[:, k * 128:(k + 1) * 128],
                            w_sb[:, rg * CO:(rg + 1) * CO],
                            start=True, stop=True,
                        )
                    src_ap = ps[:].rearrange("p (a q) -> p a q", a=2)[:, :, :2 * CO] \
                        .rearrange("p a (b f) -> p a b f", b=2)
                    dst = uh4[:, :, rg, :].rearrange("p (a b) f -> p a b f", a=2)
                    if t < 12 and rg2 == 0:
                        # DVE idles until agreement-0's first band is drained;
                        # splitting the early drains halves that latency
                        nc.vector.tensor_copy(dst, src_ap)
                    else:
                        nc.scalar.activation(dst, src_ap, mybir.ActivationFunctionType.Copy)

            if abl == 2:
                nc.vector.memset(b0[:], 0.0)

            if abl != 1:
                if abl == 0:
                    agreement(vts0, bl0)

                # ---------------- iter 1 ----------------
                softmax(b0)
                s_step()
                squash(1.0)
                if abl == 0:
                    vts1 = replicate_v()
                    agreement(vts1, bl1, ach=36)
                    nc.vector.tensor_add(b0[:], b0[:], b1[:])

                # ---------------- iter 2 ----------------
                softmax(b0)
                s_step()
                squash(1.0)
            nc.sync.dma_start(vout_d[:], v_sb[:])

    nc.compile()
    return nc


_CACHE = {}


def _get_program():
    if "nc" not in _CACHE:
        _CACHE["nc"] = _build_program()
    return _CACHE["nc"]


def _host_xbd(x_l):
    """Block-diag x, staged for DMA: [NT//STG, 128, STG*128] bf16."""
    xr = x_l.reshape(BL, RG, 16, I)
    xbd = np.zeros((NT, 128, 128), dtype=BF16_NP)
    blk = xbd.reshape(RG, NBG, 128, 128)
    for r16 in range(16):
        t = xr[:, :, r16, :]                                 # [BL, RG, I]
        t = t.transpose(1, 2, 0)                             # [RG, I, BL]
        t = t.reshape(RG, I, NBG, 8).transpose(0, 2, 1, 3)   # [RG, NBG, I, 8]
        blk[:, :, r16 * 8:(r16 + 1) * 8, r16 * 8:(r16 + 1) * 8] = t.astype(BF16_NP)
    return np.ascontiguousarray(
        xbd.reshape(NT // STG, STG, 128, 128).transpose(0, 2, 1, 3)
        .reshape(NT // STG, 128, STG * 128)
    )


def _make_in_maps(x, W):
    x = np.asarray(x, dtype=np.float32)
    W = np.asarray(W, dtype=np.float32)

    # wt[(r16, i), (rg, o, c)] — free order (o, c), c innermost
    wt = np.ascontiguousarray(
        W.reshape(RG, 16, C, O, I).transpose(1, 4, 0, 3, 2).reshape(128, RG * CO)
    ).astype(BF16_NP)
    ones_bd = np.zeros((NBG, 128, 32), dtype=BF16_NP)
    for bg in range(NBG):
        for p in range(128):
            ones_bd[bg, p, bg * 8 + p % 8] = 1.0
    rep = np.zeros((NBG, 32, 128), dtype=np.float32)
    for bg in range(NBG):
        for r16 in range(16):
            for b8 in range(8):
                rep[bg, bg * 8 + b8, r16 * 8 + b8] = 1.0

    in_maps = []
    for core in range(N_CORES):
        x_l = x[core * BL:(core + 1) * BL]
        xc = np.ascontiguousarray(
            x_l.reshape(BL, RG, 16, I).transpose(2, 3, 1, 0).reshape(128, RG * BL)
        ).astype(BF16_NP)
        in_maps.append({
            "xbd": _host_xbd(x_l),
            "wt": wt,
            "xc": xc,
            "onesbd": ones_bd,
            "rep": rep,
        })
    return in_maps


def kernel(x, W):
    in_maps = _make_in_maps(x, W)
    nc = _get_program()
    res = run_bass_kernel_spmd(nc, in_maps, list(range(N_CORES)))
    # device vout is [BL, (o, c)]; reorder to (BL, C, O)
    out = np.concatenate(
        [
            res.results[i]["vout"].reshape(BL, O, C).transpose(0, 2, 1)
            for i in range(N_CORES)
        ],
        axis=0,
    )
    return np.ascontiguousarray(out).astype(np.float32)


if __name__ == "__main__":
    rng = np.random.default_rng(0)
    x = rng.standard_normal((B, R, I), dtype=np.float32)
    W = (rng.standard_normal((R, C, O, I), dtype=np.float32) * 0.01).astype(np.float32)
    v = kernel(x=x, W=W)
    print("out", v.shape, v.dtype, np.abs(v).mean())
